# revision 9
# baseline (speedup 1.0000x reference)
"""Bilinear warp (backward-warp resampling) Trainium2 kernel.

Strategy: data-parallel over batch B=8 across the 8 NeuronCores.
Host side prepares, per batch image, a "quad-interleaved" channels-last
copy of the image in HBM: QIL[y*W + x] = 32 fp32 words =
  (x[y, x, c0..7], x[y, x+1c, c0..7], x[y+1c, x, c0..7], x[y+1c, x+1c, c0..7])
with +1c meaning clamp-to-edge. One int32 offset (= y0*W + x0) then fetches
all four bilinear neighbors for all 8 channels of one output pixel via an
indirect (gather) DMA. The device kernel computes, per output pixel,
x0/y0/wx/wy from warp, issues the gather, and does the weighted sum on the
vector engine, writing [C,H,W] tiles back to HBM.
"""
import numpy as np

import concourse.bass as bass
import concourse.bacc as bacc
import concourse.mybir as mybir
import concourse.tile as tile
from concourse.bass_utils import run_bass_kernel_spmd

B, C, H, W = 8, 8, 1024, 1024
NCORES = 8

# chunk geometry: 128 output rows x WC output cols per chunk
WC = 512
ROWCHUNKS = H // 128          # 8
COLCHUNKS = W // WC           # 2
QJ = 128                      # offsets per indirect dma call per partition? (tuned below)

_cache = {}
DBG_NO_GATHER = False
DBG_NO_STORE = False
DBG_CHUNKS = None
DBG_DUMP = False


def _build_kernel():
    """Build the Bass module (one program, runs SPMD on 8 cores)."""
    nc = bacc.Bacc("TRN2", target_bir_lowering=False, debug=False,
                   num_devices=NCORES)
    qil_d = nc.dram_tensor("qil", [H * W, 32], mybir.dt.float32,
                           kind="ExternalInput")
    warp_d = nc.dram_tensor("warp", [2, H, W], mybir.dt.float32,
                            kind="ExternalInput")
    out_d = nc.dram_tensor("out", [C, H, W], mybir.dt.float32,
                           kind="ExternalOutput")
    dbg_off_d = nc.dram_tensor("dbg_off", [128, WC], mybir.dt.int32,
                               kind="ExternalOutput") if DBG_DUMP else None
    dbg_g_d = nc.dram_tensor("dbg_g", [128, WC * 32], mybir.dt.float32,
                             kind="ExternalOutput") if DBG_DUMP else None

    with tile.TileContext(nc) as tc:
        _emit(nc, tc, qil_d, warp_d, out_d, dbg_off_d, dbg_g_d)
    nc.compile()
    return nc


def _emit(nc, tc, qil_d, warp_d, out_d, dbg_off_d, dbg_g_d):
    import contextlib
    f32 = mybir.dt.float32
    i32 = mybir.dt.int32
    with contextlib.ExitStack() as ctx:
        pool = ctx.enter_context(tc.tile_pool(name="main", bufs=2))
        gpool = ctx.enter_context(tc.tile_pool(name="big", bufs=1))
        cpool = ctx.enter_context(tc.tile_pool(name="const", bufs=1))

        # constant per-partition row index (0..127) and per-column iota
        iota_p_i = cpool.tile([128, 1], i32)
        iota_w_i = cpool.tile([128, WC], i32)
        iota_p = cpool.tile([128, 1], f32)     # partition index as float
        iota_w = cpool.tile([128, WC], f32)    # 0..WC-1 along free dim (all parts)
        nc.gpsimd.iota(iota_p_i[:], pattern=[[0, 1]], base=0, channel_multiplier=1)
        nc.gpsimd.iota(iota_w_i[:], pattern=[[1, WC]], base=0, channel_multiplier=0)
        nc.vector.tensor_copy(iota_p[:], iota_p_i[:])
        nc.vector.tensor_copy(iota_w[:], iota_w_i[:])

        chunks = [(rc, cc) for rc in range(ROWCHUNKS) for cc in range(COLCHUNKS)]
        if DBG_CHUNKS is not None:
            chunks = chunks[:DBG_CHUNKS]
        for rc, cc in chunks:
            if True:
                y0 = rc * 128
                w0 = cc * WC
                wx_t = pool.tile([128, WC], f32, tag="wx")
                wy_t = pool.tile([128, WC], f32, tag="wy")
                nc.sync.dma_start(wx_t[:], warp_d[0, y0:y0 + 128, w0:w0 + WC])
                nc.sync.dma_start(wy_t[:], warp_d[1, y0:y0 + 128, w0:w0 + WC])

                # fx = clip(wx + (w0 + iota_w), 0, W-1); fy = clip(wy + y0 + p, 0, H-1)
                fx = pool.tile([128, WC], f32, tag="fx")
                fy = pool.tile([128, WC], f32, tag="fy")
                iota_cc = pool.tile([128, WC], f32, tag="iota_cc")
                nc.vector.tensor_scalar(
                    out=iota_cc[:], in0=iota_w[:], scalar1=float(w0), scalar2=0.0,
                    op0=mybir.AluOpType.add, op1=mybir.AluOpType.bypass)
                nc.vector.tensor_add(fx[:], wx_t[:], iota_cc[:])
                nc.vector.tensor_scalar(
                    out=fx[:], in0=fx[:], scalar1=0.0, scalar2=float(W - 1),
                    op0=mybir.AluOpType.max, op1=mybir.AluOpType.min)
                rowbase = pool.tile([128, 1], f32, tag="rowbase")
                nc.vector.tensor_scalar(
                    out=rowbase[:], in0=iota_p[:], scalar1=float(y0), scalar2=0.0,
                    op0=mybir.AluOpType.add, op1=mybir.AluOpType.bypass)
                nc.vector.tensor_scalar(
                    out=fy[:], in0=wy_t[:], scalar1=rowbase[:], scalar2=0.0,
                    op0=mybir.AluOpType.add, op1=mybir.AluOpType.bypass)
                nc.vector.tensor_scalar(
                    out=fy[:], in0=fy[:], scalar1=0.0, scalar2=float(H - 1),
                    op0=mybir.AluOpType.max, op1=mybir.AluOpType.min)

                # integer parts and fractions
                x0i = pool.tile([128, WC], i32, tag="x0i")
                y0i = pool.tile([128, WC], i32, tag="y0i")
                x0f = pool.tile([128, WC], f32, tag="x0f")
                y0f = pool.tile([128, WC], f32, tag="y0f")
                # float->int conversion rounds to nearest; correct to floor
                nc.vector.tensor_copy(x0i[:], fx[:])
                nc.vector.tensor_copy(y0i[:], fy[:])
                nc.vector.tensor_copy(x0f[:], x0i[:])
                nc.vector.tensor_copy(y0f[:], y0i[:])
                gtx = pool.tile([128, WC], f32, tag="gtx")
                gty = pool.tile([128, WC], f32, tag="gty")
                nc.vector.tensor_tensor(gtx[:], x0f[:], fx[:], op=mybir.AluOpType.is_gt)
                nc.vector.tensor_tensor(gty[:], y0f[:], fy[:], op=mybir.AluOpType.is_gt)
                nc.vector.tensor_sub(x0f[:], x0f[:], gtx[:])
                nc.vector.tensor_sub(y0f[:], y0f[:], gty[:])
                nc.vector.tensor_copy(x0i[:], x0f[:])
                nc.vector.tensor_copy(y0i[:], y0f[:])
                wxf = pool.tile([128, WC], f32, tag="wxf")
                wyf = pool.tile([128, WC], f32, tag="wyf")
                nc.vector.tensor_sub(wxf[:], fx[:], x0f[:])
                nc.vector.tensor_sub(wyf[:], fy[:], y0f[:])

                # offset = y0i*W + x0i  (int32)
                off = pool.tile([128, WC], i32, tag="off")
                nc.vector.tensor_scalar(
                    out=off[:], in0=y0i[:], scalar1=W, scalar2=0,
                    op0=mybir.AluOpType.mult, op1=mybir.AluOpType.bypass)
                nc.vector.tensor_add(off[:], off[:], x0i[:])

                # gather: G[p, j, 32] = QIL[off[p, j]]
                g_t = gpool.tile([128, WC * 32], f32, tag="g")
                if DBG_NO_GATHER:
                    nc.vector.memset(g_t[:], 0.0)
                else:
                    _gather(nc, g_t, qil_d, off)

                # weights w00,w01,w10,w11 [128, WC]
                w1mx = pool.tile([128, WC], f32, tag="w1mx")
                w1my = pool.tile([128, WC], f32, tag="w1my")
                nc.vector.tensor_scalar(
                    out=w1mx[:], in0=wxf[:], scalar1=1.0, scalar2=-1.0,
                    op0=mybir.AluOpType.subtract, op1=mybir.AluOpType.mult)
                nc.vector.tensor_scalar(
                    out=w1my[:], in0=wyf[:], scalar1=1.0, scalar2=-1.0,
                    op0=mybir.AluOpType.subtract, op1=mybir.AluOpType.mult)
                wq = [pool.tile([128, WC], f32, tag=f"wq{k}", name=f"wq{k}_{rc}_{cc}")
                      for k in range(4)]
                nc.vector.tensor_mul(wq[0][:], w1mx[:], w1my[:])
                nc.vector.tensor_mul(wq[1][:], wxf[:], w1my[:])
                nc.vector.tensor_mul(wq[2][:], w1mx[:], wyf[:])
                nc.vector.tensor_mul(wq[3][:], wxf[:], wyf[:])

                # weighted sum into out_t[p, c, j]
                out_t = gpool.tile([128, C * WC], f32, tag="out")
                g3 = g_t[:].rearrange("p (j k c) -> p j k c", k=4, c=C)
                o3 = out_t[:].rearrange("p (c j) -> p c j", c=C)
                acc = pool.tile([128, WC], f32, tag="acc")
                for c in range(C):
                    nc.vector.tensor_mul(acc[:], g3[:, :, 0, c], wq[0][:])
                    for k in range(1, 4):
                        t2 = pool.tile([128, WC], f32, tag="t2")
                        nc.vector.tensor_mul(t2[:], g3[:, :, k, c], wq[k][:])
                        nc.vector.tensor_add(acc[:], acc[:], t2[:])
                    nc.vector.tensor_copy(o3[:, c, :], acc[:])

                # store: out[c, y0+p, w0:w0+WC]
                if DBG_DUMP and rc == 0 and cc == 0:
                    nc.sync.dma_start(dbg_off_d[:], off[:])
                    nc.sync.dma_start(dbg_g_d[:], g_t[:])
                if not DBG_NO_STORE:
                    nc.sync.dma_start(
                        out_d[:, y0:y0 + 128, w0:w0 + WC]
                        .rearrange("c p j -> p c j"), o3)


def _gather(nc, g_t, qil_d, off):
    """Gather 32-word quads from DRAM qil at int32 offsets off[p, j].

    indirect_dma_start consumes one offset per dest partition-row, so issue
    one call per output column: offsets off[:, j] -> G[:, j, :].
    """
    g3 = g_t[:].rearrange("p (j e) -> p j e", e=32)
    for j in range(WC):
        nc.gpsimd.indirect_dma_start(
            out=g3[:, j, :], out_offset=None, in_=qil_d[:],
            in_offset=bass.IndirectOffsetOnAxis(ap=off[:, j:j + 1], axis=0),
        )


def _host_prep(x):
    """Build per-batch quad-interleaved channels-last images [B, H*W, 32]."""
    xcl = np.ascontiguousarray(x.transpose(0, 2, 3, 1))  # [B, H, W, C]
    xr = np.concatenate([xcl[:, :, 1:, :], xcl[:, :, -1:, :]], axis=2)
    xd = np.concatenate([xcl[:, 1:, :, :], xcl[:, -1:, :, :]], axis=1)
    xdr = np.concatenate([xd[:, :, 1:, :], xd[:, :, -1:, :]], axis=2)
    qil = np.stack([xcl, xr, xd, xdr], axis=3)  # [B, H, W, 4, C]
    return np.ascontiguousarray(qil.reshape(B, H * W, 4 * C))


def kernel(x, warp):
    x = np.asarray(x, dtype=np.float32)
    warp = np.asarray(warp, dtype=np.float32)
    if "nc" not in _cache:
        _cache["nc"] = _build_kernel()
    nc = _cache["nc"]
    qil = _host_prep(x)
    ins = [{"qil": qil[b], "warp": warp[b]} for b in range(B)]
    res = run_bass_kernel_spmd(nc, ins, core_ids=list(range(NCORES)))
    out = np.stack([res.results[b]["out"] for b in range(B)], axis=0)
    return out
